# revision 35
# baseline (speedup 1.0000x reference)
"""VQ codebook (NearestEmbedEMA) Trainium2 kernel — 8 cores, batch-parallel.

Each core: 4 of 32 batches (16384 tokens x 64 dims, codebook 512).
Host prep (layout/precision only, all FLOPs on device):
  - xa    [4,2,65,4096] f16: x_aug = [x; ones] split hi/lo fp16 (exact to 2^-22)
  - xthl  [4,128,32*128] f16: x.T hi/lo pair, pre-tiled to SBUF layout
  - waug  [2,65,512] f16: [2w; -||w||^2] hi/lo fp16
  - w2    [128,512] f32: codebook replicated in both partition halves

Device, per 128-token tile:
  PE : score = xh.wh + xh.wl + xl.wh (3 accumulating fp16 matmuls, fp32 PSUM
       -> argmax-exact: 0/131072 flips vs fp32 reference)
  DVE: max over 512 codes (tensor_reduce from PSUM, per 2-tile group)
  ACT: sign(score - max) -> {0 at argmax, -1 else} (exact one-hot - 1)
  DVE: index = sum((sign >= -0.5) * iota) via scalar_tensor_tensor accum
  PE : stats += [xT_hi|xT_lo].T @ sign (accumulated in PSUM all kernel)
  (sign/index/stats of group g are emitted after group g+1's front half —
   software pipelining so no engine stalls on the ACT pass.)

Per batch: indices -> int16 wrapped-16 layout (partition-split DMAs),
gpsimd ap_gather from SBUF codebook -> quantT directly in output layout,
argmin PE-transposed + int32-converted.

Host finalize: sum per-core stats + sign-offset correction (sum of x),
n_idx = bincount(argmin), EMA/weight updates in numpy (tiny).
"""
import os
import sys
import numpy as np

sys.path.insert(0, "/opt/trn_rl_repo")

import concourse.bass as bass
import concourse.bacc as bacc
import concourse.tile as tile
from concourse import mybir
from concourse import bass_utils
from contextlib import ExitStack

f32 = mybir.dt.float32
f16 = mybir.dt.float16
i32 = mybir.dt.int32
i16 = mybir.dt.int16
u32 = mybir.dt.uint32

N_CORES = 8
B_PER_CORE = 4          # batches per core
D = 64                  # embedding dim
N_EMB = 512             # codebook size
HW = 4096               # tokens per batch (64*64)
TOK_PER_CORE = B_PER_CORE * HW   # 16384
TILE = 128              # tokens per tile
TILES_PER_B = HW // TILE         # 32
GROUP = 2               # tiles per group (psum granularity)
GROUPS_PER_B = TILES_PER_B // GROUP  # 16
DECAY = 0.99
EPS = 1e-5

_CACHED = {}


def build_kernel(nb=B_PER_CORE):
    nc = bacc.Bacc("TRN2", target_bir_lowering=False, debug=False)

    xa = nc.dram_tensor("xa", [nb, 2, 65, HW], f16, kind="ExternalInput").ap()
    xthl = nc.dram_tensor("xthl", [nb, 128, TILES_PER_B * 128], f16, kind="ExternalInput").ap()
    waug = nc.dram_tensor("waug", [2, 65, N_EMB], f16, kind="ExternalInput").ap()
    w2 = nc.dram_tensor("w2", [128, N_EMB], f32, kind="ExternalInput").ap()
    iotac = nc.dram_tensor("iotac", [1, N_EMB], f16, kind="ExternalInput").ap()

    result = nc.dram_tensor("result", [nb, D, HW], f32, kind="ExternalOutput").ap()
    argmin = nc.dram_tensor("argmin", [nb, HW], i32, kind="ExternalOutput").ap()
    stats = nc.dram_tensor("stats", [128, N_EMB], f32, kind="ExternalOutput").ap()

    with tile.TileContext(nc) as tc, ExitStack() as ctx:
        consts = ctx.enter_context(tc.tile_pool(name="consts", bufs=1))
        xa_pool = ctx.enter_context(tc.tile_pool(name="xa", bufs=2))
        xt_pool = ctx.enter_context(tc.tile_pool(name="xt", bufs=2))
        sgn_pool = ctx.enter_context(tc.tile_pool(name="sgn", bufs=2))
        sc_pool = ctx.enter_context(tc.tile_pool(name="sc", bufs=3, space="PSUM"))
        st_pool = ctx.enter_context(tc.tile_pool(name="st", bufs=1, space="PSUM"))
        tp_pool = ctx.enter_context(tc.tile_pool(name="tp", bufs=1, space="PSUM"))
        small = ctx.enter_context(tc.tile_pool(name="small", bufs=2))
        q_pool = ctx.enter_context(tc.tile_pool(name="q", bufs=2))
        out_pool = ctx.enter_context(tc.tile_pool(name="outp", bufs=2))

        # constants
        wa_sb = consts.tile([65, 2 * N_EMB], f16, tag="wa")
        nc.sync.dma_start(wa_sb[:, 0:N_EMB], waug[0])
        nc.sync.dma_start(wa_sb[:, N_EMB:2 * N_EMB], waug[1])
        iota_sb = consts.tile([128, N_EMB], f16, tag="iota")
        nc.sync.dma_start(iota_sb[:], iotac.broadcast_to((128, N_EMB)))
        w2_sb = consts.tile([128, N_EMB], f32, tag="w2")
        nc.sync.dma_start(w2_sb[:], w2[:])

        # persistent stats accumulator (PSUM): rows 0-63 hi, 64-127 lo
        st_ps = st_pool.tile([128, N_EMB], f32, tag="stats")

        from concourse import masks
        ident = consts.tile([128, 128], f32, tag="ident")
        masks.make_identity(nc, ident[:])

        n_tiles_total = nb * TILES_PER_B
        state = {"tile_idx": 0}
        pending = None

        def emit_back_half(g, sgn_sb, xt_b, idx_b):
            for t in range(GROUP):
                tg = g * GROUP + t
                if tg % 8 < 5:
                    # DVE TT at 2x + ACT accumulate: idx - sum(iota)
                    junk = sgn_pool.tile([128, N_EMB], f16, tag="junk")
                    nc.vector.tensor_tensor(
                        out=junk[:],
                        in0=sgn_sb[:, t * N_EMB:(t + 1) * N_EMB],
                        in1=iota_sb[:],
                        op=mybir.AluOpType.mult)
                    junk2 = sgn_pool.tile([128, N_EMB], f16, tag="junk2")
                    nc.scalar.activation(
                        junk2[:], junk[:],
                        mybir.ActivationFunctionType.Copy,
                        accum_out=idx_b[:, tg:tg + 1])
                else:
                    # index = sum((sign >= -0.5) * iota) on DVE (1x)
                    junk = sgn_pool.tile([128, N_EMB], f16, tag="junk")
                    nc.vector.scalar_tensor_tensor(
                        out=junk[:],
                        in0=sgn_sb[:, t * N_EMB:(t + 1) * N_EMB],
                        scalar=-0.5,
                        in1=iota_sb[:],
                        op0=mybir.AluOpType.is_ge,
                        op1=mybir.AluOpType.mult,
                        accum_out=idx_b[:, tg:tg + 1])
                # stats accumulation: [xT_hi | xT_lo].T @ sign
                ti = state["tile_idx"]
                nc.tensor.matmul(
                    st_ps[:],
                    xt_b[:, tg * 128:(tg + 1) * 128],
                    sgn_sb[:, t * N_EMB:(t + 1) * N_EMB],
                    start=(ti == 0), stop=(ti == n_tiles_total - 1),
                    skip_group_check=True)
                state["tile_idx"] = ti + 1

        loaded = {}

        def emit_epi_half(bb, idx_b, half):
            # gather chain for one partition half (tokens half*2048..+2048):
            # idx cols -> int16 -> wrapped-16 layout -> ap_gather -> store.
            tbase = half * (TILES_PER_B // 2)
            idx16h = small.tile([128, TILES_PER_B // 2], i16, tag=f"idx16{half}")
            nc.vector.tensor_copy(idx16h[:], idx_b[:, tbase:tbase + TILES_PER_B // 2])
            # full 128-partition tiles: all 8 Q7 cores run; the upper half
            # gathers index 0 (zeroed) into discarded rows -- keeps every
            # core's reads/writes inside owned SBUF.
            wraph = small.tile([128, HW // 32], i16, tag=f"wrap{half}")
            nc.gpsimd.memset(wraph[64:128, :], 0)
            for p1 in range(8):
                nc.gpsimd.dma_start(
                    wraph[0:16, :].rearrange("p (t e) -> p t e", e=8)[:, :, p1],
                    idx16h[p1 * 16:(p1 + 1) * 16, :])
            for k in range(1, 4):
                nc.gpsimd.dma_start(
                    wraph[16 * k:16 * (k + 1), :], wraph[0:16, :])
            quanth = q_pool.tile([128, HW // 2], f32, tag=f"quant{half}")
            nc.gpsimd.ap_gather(
                out_ap=quanth[:].rearrange("p (n d) -> p n d", d=1),
                in_ap=w2_sb[:].rearrange("c (n d) -> c n d", d=1),
                idxs_ap=wraph[:],
                channels=128,
                num_elems=N_EMB,
                d=1,
                num_idxs=HW // 2,
            )
            nc.sync.dma_start(
                result[bb, :, half * (HW // 2):(half + 1) * (HW // 2)],
                quanth[0:D, :])

        def emit_loads(bb):
            xa_sb = xa_pool.tile([65, 2 * HW], f16, tag="xa")
            for q in range(4):
                sl = slice(q * (HW // 4), (q + 1) * (HW // 4))
                nc.sync.dma_start(xa_sb[:, q * (HW // 4):(q + 1) * (HW // 4)], xa[bb, 0, :, sl])
                nc.sync.dma_start(
                    xa_sb[:, HW + q * (HW // 4):HW + (q + 1) * (HW // 4)], xa[bb, 1, :, sl])
            xt_b = xt_pool.tile([128, TILES_PER_B * 128], f16, tag="xt")
            for q in range(4):
                qs = TILES_PER_B * 128 // 4
                nc.sync.dma_start(
                    xt_b[:, q * qs:(q + 1) * qs], xthl[bb, :, q * qs:(q + 1) * qs])
            loaded[bb] = (xa_sb, xt_b)

        emit_loads(0)
        for b in range(nb):
            idx_b = small.tile([128, TILES_PER_B], f32, tag="idxb")
            xa_sb, xt_b = loaded.pop(b)
            for g in range(GROUPS_PER_B):
                if g == GROUPS_PER_B // 2 and b + 1 < nb:
                    emit_loads(b + 1)   # prefetch next batch mid-loop
                if g == GROUPS_PER_B // 2 + 1:
                    # ACT-path columns hold idx - sum(iota); add it back for
                    # the first half (cols 0..15) before its gather chain
                    nc.vector.tensor_scalar_add(
                        idx_b[:, 0:16].rearrange("p (a c) -> p a c", c=8)[:, :, 0:5],
                        idx_b[:, 0:16].rearrange("p (a c) -> p a c", c=8)[:, :, 0:5],
                        float(N_EMB * (N_EMB - 1) // 2))
                    emit_epi_half(b, idx_b, 0)   # first-half gather overlaps
                sc_ps = sc_pool.tile([128, GROUP * N_EMB], f32, tag="sc")
                sgn_sb = sgn_pool.tile([128, GROUP * N_EMB], f16, tag="sgn")

                for t in range(GROUP):
                    tg = g * GROUP + t   # tile within batch
                    # dist scores: (xh+xl).(wh+wl) ~ xh.wh + xh.wl + xl.wh
                    out_sl = sc_ps[:, t * N_EMB:(t + 1) * N_EMB]
                    xh = xa_sb[:, tg * TILE:(tg + 1) * TILE]
                    xl = xa_sb[:, HW + tg * TILE:HW + (tg + 1) * TILE]
                    wh = wa_sb[:, 0:N_EMB]
                    wl = wa_sb[:, N_EMB:2 * N_EMB]
                    nc.tensor.matmul(out_sl, xh, wh, start=True, stop=False,
                                     skip_group_check=True)
                    nc.tensor.matmul(out_sl, xh, wl, start=False, stop=False,
                                     skip_group_check=True)
                    nc.tensor.matmul(out_sl, xl, wh, start=False, stop=True,
                                     skip_group_check=True)

                # max over codes: [128, GROUP, 512] -> [128, GROUP]
                mx = small.tile([128, GROUP], f32, tag="mx")
                nc.vector.tensor_reduce(
                    mx[:], sc_ps[:].rearrange("p (g n) -> p g n", n=N_EMB),
                    axis=mybir.AxisListType.X, op=mybir.AluOpType.max)
                nmx = small.tile([128, GROUP], f32, tag="nmx")
                nc.vector.tensor_scalar_mul(nmx[:], mx[:], -1.0)

                for t in range(GROUP):
                    tg = g * GROUP + t
                    # onehot-ish: sign(score - max) in {-1, 0}
                    nc.scalar.activation(
                        sgn_sb[:, t * N_EMB:(t + 1) * N_EMB],
                        sc_ps[:, t * N_EMB:(t + 1) * N_EMB],
                        mybir.ActivationFunctionType.Sign,
                        bias=nmx[:, t:t + 1], scale=1.0)

                # back half of the previous group (software pipelining: lets
                # DVE/PE proceed without stalling on this group's ACT signs)
                if pending is not None:
                    emit_back_half(*pending)
                pending = (g, sgn_sb, xt_b, idx_b)
            emit_back_half(*pending)
            pending = None

            # ---- per-batch epilogue (second half + argmin out) ----
            nc.vector.tensor_scalar_add(
                idx_b[:, 16:32].rearrange("p (a c) -> p a c", c=8)[:, :, 0:5],
                idx_b[:, 16:32].rearrange("p (a c) -> p a c", c=8)[:, :, 0:5],
                float(N_EMB * (N_EMB - 1) // 2))
            emit_epi_half(b, idx_b, 1)
            idxt_ps = tp_pool.tile([TILES_PER_B, 128], f32, tag="tp")
            nc.tensor.transpose(idxt_ps[:], idx_b[:], ident[:])
            idxt_sb = out_pool.tile([TILES_PER_B, 128], i32, tag="idxt_sb")
            nc.scalar.copy(idxt_sb[:], idxt_ps[:])
            nc.sync.dma_start(
                argmin[b].rearrange("(t p) -> t p", p=128), idxt_sb[:])

        # stats out
        st_sb = out_pool.tile([128, N_EMB], f32, tag="st_sb")
        nc.scalar.copy(st_sb[:], st_ps[:])
        nc.sync.dma_start(stats[:], st_sb[:])

    nc.compile()
    return nc


def _hl16(a):
    hi = a.astype(np.float16)
    lo = (a - hi.astype(np.float32)).astype(np.float16)
    return hi, lo


def _prep_shared(weight):
    w = np.asarray(weight, dtype=np.float32)
    wn = (w.astype(np.float64) ** 2).sum(axis=0)
    waug = np.concatenate([2.0 * w, -wn[None, :].astype(np.float32)], axis=0)
    waug = waug.astype(np.float32)
    wh, wl = _hl16(waug)
    waug16 = np.ascontiguousarray(np.stack([wh, wl], axis=0))   # [2, 65, 512]
    w2 = np.concatenate([w, w], axis=0)
    iotac = np.arange(N_EMB, dtype=np.float16)[None, :]
    return waug16, w2, iotac


def kernel(x, weight, cluster_size, embed_avg):
    x = np.asarray(x, dtype=np.float32)
    weight = np.asarray(weight, dtype=np.float32)
    cluster_size = np.asarray(cluster_size, dtype=np.float32)
    embed_avg = np.asarray(embed_avg, dtype=np.float32)

    B, D_, H, W = x.shape
    assert (B, D_, H, W) == (32, 64, 64, 64)

    if "nc" not in _CACHED:
        _CACHED["nc"] = build_kernel()
    nc = _CACHED["nc"]

    waug, w2, iotac = _prep_shared(weight)

    xr = x.reshape(B, D_, HW)                        # [32, 64, 4096]
    ones = np.ones((1, HW), np.float32)
    in_maps = []
    for c in range(N_CORES):
        xb = xr[c * B_PER_CORE:(c + 1) * B_PER_CORE]          # [4, 64, 4096]
        xa32 = np.concatenate([xb, np.broadcast_to(ones, (B_PER_CORE, 1, HW))], axis=1)
        xah, xal = _hl16(np.ascontiguousarray(xa32, dtype=np.float32))
        xa = np.ascontiguousarray(np.stack([xah, xal], axis=1))  # [4, 2, 65, 4096]
        xt = np.ascontiguousarray(xb.transpose(0, 2, 1))      # [4, 4096, 64]
        xt_hi = xt.astype(np.float16)
        xt_lo = (xt - xt_hi.astype(np.float32)).astype(np.float16)
        xthl = np.concatenate([xt_hi, xt_lo], axis=2)         # [4, 4096, 128]
        # device layout: [128 partitions(token%128), tile, col]
        xthl = np.ascontiguousarray(
            xthl.reshape(B_PER_CORE, TILES_PER_B, 128, 128)
                .transpose(0, 2, 1, 3)
                .reshape(B_PER_CORE, 128, TILES_PER_B * 128))
        in_maps.append({
            "xa": xa, "xthl": xthl, "waug": waug, "w2": w2, "iotac": iotac,
            
        })

    res = bass_utils.run_bass_kernel_spmd(
        nc, in_maps, list(range(N_CORES)),
        trace=bool(os.environ.get("VQ_TRACE")))
    _CACHED["last_results"] = res

    # ---- host finalize ----
    result = np.empty((B, D_, HW), np.float32)
    am = np.empty((B, HW), np.int32)
    stats_sum = np.zeros((128, N_EMB), np.float64)
    corr = np.zeros((D_,), np.float64)
    for c in range(N_CORES):
        r = res.results[c]
        result[c * B_PER_CORE:(c + 1) * B_PER_CORE] = r["result"]
        am[c * B_PER_CORE:(c + 1) * B_PER_CORE] = r["argmin"]
        stats_sum += r["stats"].astype(np.float64)
        # correction: stats accumulated (onehot - 1) -> add sum over tokens of
        # (xt_hi + xt_lo) per dim
        xthl = in_maps[c]["xthl"].astype(np.float64)
        corr += xthl.reshape(-1, 128).sum(axis=0)[:D_] + \
            xthl.reshape(-1, 128).sum(axis=0)[D_:]

    embed_sum = (stats_sum[:D_] + stats_sum[D_:] + corr[:, None]).astype(np.float32)

    flat_am = am.reshape(-1)
    n_idx = np.bincount(flat_am, minlength=N_EMB).astype(np.float32)
    n_idx = np.where(n_idx == 0, np.float32(1.0), n_idx)

    new_cluster_size = DECAY * cluster_size + (1.0 - DECAY) * n_idx
    new_embed_avg = DECAY * embed_avg + (1.0 - DECAY) * embed_sum
    n = new_cluster_size.sum(dtype=np.float32)
    cs = (new_cluster_size + EPS) / (n + N_EMB * EPS) * n
    new_weight = new_embed_avg / cs[None, :]

    result4 = result.reshape(B, D_, H, W)
    am3 = flat_am.reshape(B, H, W).astype(np.int32)
    return (result4, am3, new_weight.astype(np.float32),
            new_cluster_size.astype(np.float32), new_embed_avg.astype(np.float32))
